# revision 17
# baseline (speedup 1.0000x reference)
"""Trainium2 Bass kernel for MemoryModule (cosine-similarity retrieval).

out = x + memory[argmax_m cos_sim(x, memory)]

Strategy (8 NeuronCores, data-parallel over batch):
  - Each core gets B/8 = 1024 rows of x, full memory table (replicated).
  - Coarse pass: sim = x @ (memory * inv_norm)^T on TensorE in float32r
    (11-bit mantissa, full 1 cyc/row rate).  x-row norms cancel in argmax so
    x is never normalized.  Sim stored bf16 in SBUF per 4096-col block;
    VectorE max/max_index extract per-block top-8 candidates.
  - Merge per-block candidates to global top-8 per row (onehot arithmetic).
  - Exact pass: gather the 8 candidate memory rows (indirect DMA), rescore
    in fp32 (exact dots + Newton-refined inv-norms), pick the true argmax,
    add the winning row to x.
"""

import numpy as np

import concourse.bass as bass
import concourse.mybir as mybir
import concourse.tile as tile
from concourse import bacc, bass_utils
from concourse.bass_interp import get_hw_module
from concourse.masks import make_identity

dt = mybir.dt
AF = mybir.ActivationFunctionType
ALU = mybir.AluOpType

B, D, M, NCORES = 8192, 512, 32768, 8


def build(B_loc=1024, M_=32768, D_=512, num_devices=NCORES):
    BT = B_loc // 128       # batch tiles per core
    KT = D_ // 128          # contraction tiles (4)
    MC = M_ // 512          # m-chunks (64)
    BLK = 2048              # m per scan block
    CPB = BLK // 512        # chunks per block (8)
    NBLK = M_ // BLK        # scan blocks (8)
    NCAND = NBLK * 8        # per-row candidates before merge
    NT = M_ // 128          # norm tiles (256)
    assert MC <= 128

    nc = bacc.Bacc("TRN2", target_bir_lowering=False, debug=False,
                   num_devices=num_devices)
    x = nc.dram_tensor("x", [B_loc, D_], dt.float32, kind="ExternalInput").ap()
    xT = nc.dram_tensor("xT", [D_, B_loc], dt.float32, kind="ExternalInput").ap()
    mem = nc.dram_tensor("mem", [M_, D_], dt.float32, kind="ExternalInput").ap()
    memT = nc.dram_tensor("memT", [D_, M_], dt.float32, kind="ExternalInput").ap()
    iota = nc.dram_tensor("iota", [128, NCAND], dt.float32, kind="ExternalInput").ap()
    blkoff = nc.dram_tensor("blkoff", [128, NCAND], dt.float32, kind="ExternalInput").ap()
    out = nc.dram_tensor("out", [B_loc, D_], dt.float32, kind="ExternalOutput").ap()

    with tile.TileContext(nc) as tc:
        with (
            tc.tile_pool(name="pp", bufs=1) as pp,
            tc.tile_pool(name="wp", bufs=2) as wp,
            tc.tile_pool(name="ps_mm", bufs=4, space="PSUM") as ps_mm,
            tc.tile_pool(name="ps_b", bufs=2, space="PSUM") as ps_b,
            tc.tile_pool(name="dp", bufs=1, space="DRAM") as dp,
        ):
            # ---- constants
            iota_t = pp.tile([128, NCAND], dt.float32, name="iota_t")
            nc.sync.dma_start(out=iota_t[:], in_=iota[:])
            blkoff_t = pp.tile([128, NCAND], dt.float32, name="blkoff_t")
            nc.sync.dma_start(out=blkoff_t[:], in_=blkoff[:])
            ones_t = pp.tile([1, 128], dt.float32, name="ones_t")
            nc.vector.memset(ones_t[:], 1.0)

            # ---- xT loaded and rounded to fp32r
            xTr = []
            for k in range(KT):
                xraw = wp.tile([128, B_loc], dt.float32, name="xraw", tag="xraw", bufs=1)
                nc.sync.dma_start(out=xraw[:], in_=xT[k * 128:(k + 1) * 128, :])
                xr = pp.tile([128, B_loc], dt.float32r, name=f"xTr{k}", tag=f"xTr{k}")
                nc.vector.tensor_copy(xr[:], xraw[:])
                xTr.append(xr)

            # ---- memory row norms -> inv_norm, linearized to DRAM and back
            nsq = pp.tile([128, NT], dt.float32, name="nsq")
            for t in range(NT):
                mtile = wp.tile([128, D_], dt.float32, name="mtile", tag="mtile", bufs=3)
                nc.sync.dma_start(out=mtile[:], in_=mem[t * 128:(t + 1) * 128, :])
                sq_s = wp.tile([128, D_], dt.float32, name="sq_s", tag="sq_s", bufs=2)
                nc.scalar.activation(out=sq_s[:], in_=mtile[:], func=AF.Square,
                                     accum_out=nsq[:, t:t + 1])
            nrm = pp.tile([128, NT], dt.float32, name="nrm")
            nc.scalar.sqrt(nrm[:], nsq[:])
            inv = pp.tile([128, NT], dt.float32, name="inv")
            nc.vector.reciprocal(inv[:], nrm[:])

            # Linearize inv ([128, NT], elem (p,t) = inv_norm[t*128+p]) to DRAM
            # m-order via PE transposes + contiguous DMA.
            ident = pp.tile([128, 128], dt.float32, name="ident")
            make_identity(nc, ident[:])
            dinv = dp.tile([NT, 128], dt.float32, name="dinv")
            for h in range((NT + 127) // 128):
                w = min(128, NT - h * 128)
                invp = ps_b.tile([128, 128], dt.float32, name="invp", tag="invp")
                nc.tensor.transpose(invp[:w, :], inv[:, h * 128:h * 128 + w], ident[:])
                invts = wp.tile([128, 128], dt.float32, name="invts", tag="invts", bufs=2)
                nc.vector.tensor_copy(invts[:w, :], invp[:w, :])
                nc.sync.dma_start(out=dinv[h * 128:h * 128 + w, :], in_=invts[:w, :])
            dinv_lin = dinv[:].rearrange("(b a) c -> b (a c)", b=1)  # [1, M_] view

            # ---- persistent per-btile candidate buffers
            valb = [pp.tile([128, NCAND], dt.bfloat16, name=f"valb{bt}") for bt in range(BT)]
            idxu = [pp.tile([128, NCAND], dt.uint32, name=f"idxu{bt}") for bt in range(BT)]

            # ---- coarse pass
            for q in range(NBLK):
                sims = []
                for bt in range(BT):
                    s = wp.tile([128, BLK], dt.bfloat16, name=f"sim{bt}", tag=f"sim{bt}", bufs=2)
                    sims.append(s)
                for cc in range(CPB):
                    c = q * CPB + cc
                    # broadcast inv_norm[c*512:(c+1)*512] to 128 partitions (K=1 matmul)
                    invc = wp.tile([1, 512], dt.float32, name="invc", tag="invc", bufs=2)
                    nc.sync.dma_start(out=invc[:], in_=dinv_lin[0:1, c * 512:(c + 1) * 512])
                    bc = ps_b.tile([128, 512], dt.float32, name="bc", tag="bc")
                    nc.tensor.matmul(bc[:], ones_t[0:1, :], invc[0:1, :],
                                     start=True, stop=True)
                    mtsr = []
                    for k in range(KT):
                        mraw = wp.tile([128, 512], dt.float32, name="mraw", tag="mraw", bufs=3)
                        nc.sync.dma_start(out=mraw[:],
                                          in_=memT[k * 128:(k + 1) * 128, c * 512:(c + 1) * 512])
                        msc = wp.tile([128, 512], dt.float32r, name="msc", tag="msc", bufs=8)
                        nc.vector.tensor_tensor(out=msc[:], in0=mraw[:], in1=bc[:], op=ALU.mult)
                        mtsr.append(msc)
                    for bt in range(BT):
                        pm = ps_mm.tile([128, 512], dt.float32, name="pm", tag="pm")
                        for k in range(KT):
                            nc.tensor.matmul(pm[:], xTr[k][:, bt * 128:(bt + 1) * 128],
                                             mtsr[k][:], start=(k == 0), stop=(k == KT - 1))
                        nc.scalar.copy(sims[bt][:, cc * 512:(cc + 1) * 512], pm[:])
                for bt in range(BT):
                    nc.vector.max(out=valb[bt][:, q * 8:(q + 1) * 8], in_=sims[bt][:])
                    nc.vector.max_index(out=idxu[bt][:, q * 8:(q + 1) * 8],
                                        in_max=valb[bt][:, q * 8:(q + 1) * 8],
                                        in_values=sims[bt][:])

            # ---- merge + exact rescore per btile
            for bt in range(BT):
                idxf = wp.tile([128, NCAND], dt.float32, name="idxf", tag="idxf", bufs=2)
                nc.vector.tensor_copy(idxf[:], idxu[bt][:])
                nc.vector.tensor_tensor(out=idxf[:], in0=idxf[:], in1=blkoff_t[:], op=ALU.add)

                m8 = wp.tile([128, 8], dt.bfloat16, name="m8", tag="m8", bufs=2)
                nc.vector.max(out=m8[:], in_=valb[bt][:])
                p8 = wp.tile([128, 8], dt.uint32, name="p8", tag="p8", bufs=2)
                nc.vector.max_index(out=p8[:], in_max=m8[:], in_values=valb[bt][:])
                p8f = wp.tile([128, 8], dt.float32, name="p8f", tag="p8f", bufs=2)
                nc.vector.tensor_copy(p8f[:], p8[:])

                # gather global fp32 index of each of the 8 merged candidates
                idx8 = wp.tile([128, 8], dt.float32, name="idx8", tag="idx8", bufs=2)
                for j in range(8):
                    msk = wp.tile([128, NCAND], dt.float32, name="msk", tag="msk", bufs=2)
                    nc.vector.tensor_scalar(msk[:], iota_t[:], p8f[:, j:j + 1], None,
                                            op0=ALU.is_equal)
                    nc.vector.tensor_tensor(out=msk[:], in0=msk[:], in1=idxf[:], op=ALU.mult)
                    nc.vector.tensor_reduce(out=idx8[:, j:j + 1], in_=msk[:],
                                            axis=mybir.AxisListType.X, op=ALU.add)
                idx8u = wp.tile([128, 8], dt.uint32, name="idx8u", tag="idx8u", bufs=2)
                nc.vector.tensor_copy(idx8u[:], idx8[:])

                xbt = wp.tile([128, D_], dt.float32, name="xbt", tag="xbt", bufs=2)
                nc.sync.dma_start(out=xbt[:], in_=x[bt * 128:(bt + 1) * 128, :])

                nsq8 = wp.tile([128, 8], dt.float32, name="nsq8", tag="nsq8", bufs=2)
                dot8 = wp.tile([128, 8], dt.float32, name="dot8", tag="dot8", bufs=2)
                cands = []
                for j in range(8):
                    cj = wp.tile([128, D_], dt.float32, name=f"cand{j}", tag=f"cand{j}", bufs=1)
                    nc.gpsimd.indirect_dma_start(
                        out=cj[:], out_offset=None, in_=mem[:],
                        in_offset=bass.IndirectOffsetOnAxis(ap=idx8u[:, j:j + 1], axis=0))
                    cands.append(cj)
                    csq = wp.tile([128, D_], dt.float32, name="csq", tag="sq_s", bufs=2)
                    nc.scalar.activation(out=csq[:], in_=cj[:], func=AF.Square,
                                         accum_out=nsq8[:, j:j + 1])
                    cprod = wp.tile([128, D_], dt.float32, name="cprod", tag="sq_s", bufs=2)
                    nc.vector.tensor_tensor(out=cprod[:], in0=cj[:], in1=xbt[:], op=ALU.mult)
                    nc.vector.tensor_reduce(out=dot8[:, j:j + 1], in_=cprod[:],
                                            axis=mybir.AxisListType.X, op=ALU.add)

                # inv_norm = 1/sqrt(nsq8), one Newton step for fp32-grade accuracy
                nrm8 = wp.tile([128, 8], dt.float32, name="nrm8", tag="nrm8", bufs=2)
                nc.scalar.sqrt(nrm8[:], nsq8[:])
                y0 = wp.tile([128, 8], dt.float32, name="y0", tag="y0", bufs=2)
                nc.vector.reciprocal(y0[:], nrm8[:])
                t1 = wp.tile([128, 8], dt.float32, name="t1", tag="t1", bufs=2)
                nc.vector.tensor_tensor(out=t1[:], in0=y0[:], in1=y0[:], op=ALU.mult)
                nc.vector.tensor_tensor(out=t1[:], in0=t1[:], in1=nsq8[:], op=ALU.mult)
                nc.vector.tensor_scalar(t1[:], t1[:], -0.5, 1.5, op0=ALU.mult, op1=ALU.add)
                nc.vector.tensor_tensor(out=t1[:], in0=t1[:], in1=y0[:], op=ALU.mult)

                sc8 = wp.tile([128, 8], dt.float32, name="sc8", tag="sc8", bufs=2)
                nc.vector.tensor_tensor(out=sc8[:], in0=dot8[:], in1=t1[:], op=ALU.mult)

                mx8 = wp.tile([128, 8], dt.float32, name="mx8", tag="mx8", bufs=2)
                nc.vector.max(out=mx8[:], in_=sc8[:])
                si = wp.tile([128, 8], dt.uint32, name="si", tag="si", bufs=2)
                nc.vector.max_index(out=si[:], in_max=mx8[:], in_values=sc8[:])
                sif = wp.tile([128, 1], dt.float32, name="sif", tag="sif", bufs=2)
                nc.vector.tensor_copy(sif[:], si[:, 0:1])

                for j in range(8):
                    mjf = wp.tile([128, 1], dt.float32, name="mjf", tag="mjf", bufs=2)
                    nc.vector.tensor_scalar(mjf[:], sif[:], float(j), None, op0=ALU.is_equal)
                    nc.vector.scalar_tensor_tensor(out=xbt[:], in0=cands[j][:],
                                                   scalar=mjf[:], in1=xbt[:],
                                                   op0=ALU.mult, op1=ALU.add)
                nc.sync.dma_start(out=out[bt * 128:(bt + 1) * 128, :], in_=xbt[:])

    nc.compile()
    return nc


def make_in_maps(x, memory, B_loc=None, M_=None, D_=None, ncores=NCORES):
    Bfull, D_ = x.shape
    M_, _ = memory.shape
    if B_loc is None:
        B_loc = Bfull // ncores
    BLK = 2048
    NBLK = M_ // BLK
    NCAND = NBLK * 8
    iota = np.broadcast_to(np.arange(NCAND, dtype=np.float32), (128, NCAND)).copy()
    blkoff = np.broadcast_to(
        (np.arange(NCAND, dtype=np.float32) // 8).astype(np.float32) * BLK,
        (128, NCAND)).copy()
    memT = np.ascontiguousarray(memory.T)
    mem = np.ascontiguousarray(memory)
    in_maps = []
    for i in range(ncores):
        xs = np.ascontiguousarray(x[i * B_loc:(i + 1) * B_loc])
        in_maps.append(dict(x=xs, xT=np.ascontiguousarray(xs.T), mem=mem,
                            memT=memT, iota=iota, blkoff=blkoff))
    return in_maps


_cached = {}


def kernel(x, memory):
    x = np.asarray(x, dtype=np.float32)
    memory = np.asarray(memory, dtype=np.float32)
    Bfull, D_ = x.shape
    M_, _ = memory.shape
    B_loc = Bfull // NCORES
    key = (B_loc, M_, D_)
    if key not in _cached:
        nc = build(B_loc=B_loc, M_=M_, D_=D_)
        nc.m = get_hw_module(nc.m)
        _cached[key] = nc
    nc = _cached[key]
    in_maps = make_in_maps(x, memory, B_loc=B_loc, M_=M_, D_=D_)
    res = bass_utils.run_bass_kernel_spmd(nc, in_maps, core_ids=list(range(NCORES)))
    return np.concatenate([res.results[i]["out"] for i in range(NCORES)], axis=0)


# revision 26
# speedup vs baseline: 1.4230x; 1.4230x over previous
"""Trainium2 Bass kernel for MemoryModule (cosine-similarity retrieval).

out = x + memory[argmax_m cos_sim(x, memory)]

Strategy (8 NeuronCores, data-parallel over batch):
  - Each core gets B/8 = 1024 rows of x, full memory table (replicated).
  - Coarse pass: sim = x @ (memory * inv_norm)^T on TensorE in float32r
    (11-bit mantissa, full 1 cyc/row rate).  x-row norms cancel in argmax so
    x is never normalized.  Sim stored bf16 in SBUF per 4096-col block;
    VectorE max/max_index extract per-block top-8 candidates.
  - Merge per-block candidates to global top-8 per row (onehot arithmetic).
  - Exact pass: gather the 8 candidate memory rows (indirect DMA), rescore
    in fp32 (exact dots + Newton-refined inv-norms), pick the true argmax,
    add the winning row to x.
"""

import numpy as np

import concourse.bass as bass
import concourse.mybir as mybir
import concourse.tile as tile
from concourse import bacc, bass_utils
from concourse.bass_interp import get_hw_module
from concourse.masks import make_identity

dt = mybir.dt
AF = mybir.ActivationFunctionType
ALU = mybir.AluOpType

B, D, M, NCORES = 8192, 512, 32768, 8


def build(B_loc=1024, M_=32768, D_=512, num_devices=NCORES):
    BT = B_loc // 128       # batch tiles per core
    KT = D_ // 128          # contraction tiles (4)
    MC = M_ // 512          # m-chunks (64)
    BLK = 2048              # m per scan block
    CPB = BLK // 512        # chunks per block (8)
    NBLK = M_ // BLK        # scan blocks (8)
    NCAND = NBLK * 8        # per-row candidates before merge
    NT = M_ // 128          # norm tiles (256)
    assert MC <= 128

    nc = bacc.Bacc("TRN2", target_bir_lowering=False, debug=False,
                   num_devices=num_devices)
    x = nc.dram_tensor("x", [B_loc, D_], dt.float32, kind="ExternalInput").ap()
    xT = nc.dram_tensor("xT", [D_, B_loc], dt.float32, kind="ExternalInput").ap()
    mem = nc.dram_tensor("mem", [M_, D_], dt.float32, kind="ExternalInput").ap()
    memT = nc.dram_tensor("memT", [D_, M_], dt.float32, kind="ExternalInput").ap()
    iota = nc.dram_tensor("iota", [128, NCAND], dt.float32, kind="ExternalInput").ap()
    blkoff = nc.dram_tensor("blkoff", [128, NCAND], dt.float32, kind="ExternalInput").ap()
    out = nc.dram_tensor("out", [B_loc, D_], dt.float32, kind="ExternalOutput").ap()

    with tile.TileContext(nc) as tc:
        with (
            tc.tile_pool(name="pp", bufs=1) as pp,
            tc.tile_pool(name="wp", bufs=2) as wp,
            tc.tile_pool(name="ps_mm", bufs=6, space="PSUM") as ps_mm,
            tc.tile_pool(name="ps_b", bufs=2, space="PSUM") as ps_b,
            tc.tile_pool(name="dp", bufs=1, space="DRAM") as dp,
        ):
            # ---- constants
            iota_t = pp.tile([128, NCAND], dt.float32, name="iota_t")
            nc.sync.dma_start(out=iota_t[:], in_=iota[:])
            blkoff_t = pp.tile([128, NCAND], dt.float32, name="blkoff_t")
            nc.sync.dma_start(out=blkoff_t[:], in_=blkoff[:])
            ones_t = pp.tile([1, 128], dt.float32, name="ones_t")
            nc.vector.memset(ones_t[:], 1.0)

            # ---- xT loaded and rounded to fp32r
            xTr = []
            for k in range(KT):
                xraw = wp.tile([128, B_loc], dt.float32, name="xraw", tag="xraw", bufs=1)
                nc.sync.dma_start(out=xraw[:], in_=xT[k * 128:(k + 1) * 128, :])
                xr = pp.tile([128, B_loc], dt.float32r, name=f"xTr{k}", tag=f"xTr{k}")
                nc.vector.tensor_copy(xr[:], xraw[:])
                xTr.append(xr)

            # ---- memory row norms -> inv_norm, linearized to DRAM and back
            nsq = pp.tile([128, NT], dt.float32, name="nsq")
            for t in range(NT):
                mtile = wp.tile([128, D_], dt.float32, name="mtile", tag="mtile", bufs=3)
                nc.sync.dma_start(out=mtile[:], in_=mem[t * 128:(t + 1) * 128, :])
                sq_s = wp.tile([128, D_], dt.float32, name="sq_s", tag="sq_s", bufs=2)
                nc.scalar.activation(out=sq_s[:], in_=mtile[:], func=AF.Square,
                                     accum_out=nsq[:, t:t + 1])
            nrm = pp.tile([128, NT], dt.float32, name="nrm")
            nc.scalar.sqrt(nrm[:], nsq[:])
            inv = pp.tile([128, NT], dt.float32, name="inv")
            nc.vector.reciprocal(inv[:], nrm[:])

            # Linearize inv ([128, NT], elem (p,t) = inv_norm[t*128+p]) to DRAM
            # m-order via PE transposes + contiguous DMA.
            ident = pp.tile([128, 128], dt.float32, name="ident")
            make_identity(nc, ident[:])
            NH = (NT + 127) // 128
            dinvs, dinv_lins = [], []
            for h in range(NH):
                w = min(128, NT - h * 128)
                dinv = dp.tile([w, 128], dt.float32, name=f"dinv{h}", tag=f"dinv{h}")
                invp = ps_b.tile([128, 128], dt.float32, name="invp", tag="bc")
                nc.tensor.transpose(invp[:w, :], inv[:, h * 128:h * 128 + w], ident[:])
                invts = wp.tile([128, 128], dt.float32, name="invts", tag="invts", bufs=2)
                nc.vector.tensor_copy(invts[:w, :], invp[:w, :])
                nc.sync.dma_start(out=dinv[:], in_=invts[:w, :])
                dinvs.append(dinv)
                dinv_lins.append(dinv[:].rearrange("(b a) c -> b (a c)", b=1))
            CPH = (128 * 128) // 512  # m-chunks covered per dinv tile (32)

            def dinv_slice(c):
                h = c // CPH
                lo = (c - h * CPH) * 512
                return dinv_lins[h][0:1, lo:lo + 512]

            # ---- persistent per-btile candidate buffers
            valb = [pp.tile([128, NCAND], dt.bfloat16, name=f"valb{bt}") for bt in range(BT)]
            idxu = [pp.tile([128, NCAND], dt.uint32, name=f"idxu{bt}") for bt in range(BT)]

            # ---- coarse pass
            for q in range(NBLK):
                sims = []
                for bt in range(BT):
                    s = wp.tile([128, BLK], dt.bfloat16, name=f"sim{bt}", tag=f"sim{bt}", bufs=2)
                    sims.append(s)
                for cc in range(CPB):
                    c = q * CPB + cc
                    # broadcast inv_norm[c*512:(c+1)*512] to 128 partitions (K=1 matmul)
                    invc = wp.tile([1, 512], dt.float32, name="invc", tag="invc", bufs=2)
                    nc.sync.dma_start(out=invc[:], in_=dinv_slice(c))
                    bc = ps_b.tile([128, 512], dt.float32, name="bc", tag="bc")
                    nc.tensor.matmul(bc[:], ones_t[0:1, :], invc[0:1, :],
                                     start=True, stop=True)
                    bcs = wp.tile([128, 512], dt.float32, name="bcs", tag="bcs", bufs=2)
                    nc.scalar.copy(bcs[:], bc[:])
                    mtsr = []
                    for k in range(KT):
                        mraw = wp.tile([128, 512], dt.float32, name="mraw", tag="mraw", bufs=3)
                        nc.sync.dma_start(out=mraw[:],
                                          in_=memT[k * 128:(k + 1) * 128, c * 512:(c + 1) * 512])
                        msc = wp.tile([128, 512], dt.float32r, name="msc", tag="msc", bufs=8)
                        nc.gpsimd.tensor_tensor(out=msc[:], in0=mraw[:], in1=bcs[:], op=ALU.mult)
                        mtsr.append(msc)
                    for bt in range(BT):
                        pm = ps_mm.tile([128, 512], dt.float32, name="pm", tag="pm")
                        for k in range(KT):
                            nc.tensor.matmul(pm[:], xTr[k][:, bt * 128:(bt + 1) * 128],
                                             mtsr[k][:], start=(k == 0), stop=(k == KT - 1))
                        nc.scalar.copy(sims[bt][:, cc * 512:(cc + 1) * 512], pm[:])
                for bt in range(BT):
                    nc.vector.max(out=valb[bt][:, q * 8:(q + 1) * 8], in_=sims[bt][:])
                    nc.vector.max_index(out=idxu[bt][:, q * 8:(q + 1) * 8],
                                        in_max=valb[bt][:, q * 8:(q + 1) * 8],
                                        in_values=sims[bt][:])

            # ---- merge + exact rescore per btile
            for bt in range(BT):
                idxf = wp.tile([128, NCAND], dt.float32, name="idxf", tag="idxf", bufs=2)
                nc.vector.tensor_copy(idxf[:], idxu[bt][:])
                nc.vector.tensor_tensor(out=idxf[:], in0=idxf[:], in1=blkoff_t[:], op=ALU.add)

                m8 = wp.tile([128, 8], dt.bfloat16, name="m8", tag="m8", bufs=2)
                nc.vector.max(out=m8[:], in_=valb[bt][:])
                p8 = wp.tile([128, 8], dt.uint32, name="p8", tag="p8", bufs=2)
                nc.vector.max_index(out=p8[:], in_max=m8[:], in_values=valb[bt][:])
                p8f = wp.tile([128, 8], dt.float32, name="p8f", tag="p8f", bufs=2)
                nc.vector.tensor_copy(p8f[:], p8[:])

                # gather global fp32 index of each of the 8 merged candidates
                idx8 = wp.tile([128, 8], dt.float32, name="idx8", tag="idx8", bufs=2)
                for j in range(8):
                    msk = wp.tile([128, NCAND], dt.float32, name="msk", tag="msk", bufs=2)
                    nc.vector.scalar_tensor_tensor(out=msk[:], in0=iota_t[:],
                                                   scalar=p8f[:, j:j + 1], in1=idxf[:],
                                                   op0=ALU.is_equal, op1=ALU.mult,
                                                   accum_out=idx8[:, j:j + 1])
                idx8u = wp.tile([128, 8], dt.uint32, name="idx8u", tag="idx8u", bufs=2)
                nc.vector.tensor_copy(idx8u[:], idx8[:])

                xbt = wp.tile([128, D_], dt.float32, name="xbt", tag="xbt", bufs=2)
                nc.sync.dma_start(out=xbt[:], in_=x[bt * 128:(bt + 1) * 128, :])

                nsq8 = wp.tile([128, 8], dt.float32, name="nsq8", tag="nsq8", bufs=2)
                dot8 = wp.tile([128, 8], dt.float32, name="dot8", tag="dot8", bufs=2)
                cands = []
                for j in range(8):
                    cj = wp.tile([128, D_], dt.float32, name=f"cand{j}", tag=f"cand{j}", bufs=1)
                    nc.gpsimd.indirect_dma_start(
                        out=cj[:], out_offset=None, in_=mem[:],
                        in_offset=bass.IndirectOffsetOnAxis(ap=idx8u[:, j:j + 1], axis=0))
                    cands.append(cj)
                    csq = wp.tile([128, D_], dt.float32, name="csq", tag="sq_s", bufs=2)
                    nc.scalar.activation(out=csq[:], in_=cj[:], func=AF.Square,
                                         accum_out=nsq8[:, j:j + 1])
                    cprod = wp.tile([128, D_], dt.float32, name="cprod", tag="sq_s", bufs=2)
                    nc.vector.scalar_tensor_tensor(out=cprod[:], in0=cj[:], scalar=1.0,
                                                   in1=xbt[:], op0=ALU.mult, op1=ALU.mult,
                                                   accum_out=dot8[:, j:j + 1])

                # inv_norm = 1/sqrt(nsq8), one Newton step for fp32-grade accuracy
                nrm8 = wp.tile([128, 8], dt.float32, name="nrm8", tag="nrm8", bufs=2)
                nc.scalar.sqrt(nrm8[:], nsq8[:])
                y0 = wp.tile([128, 8], dt.float32, name="y0", tag="y0", bufs=2)
                nc.vector.reciprocal(y0[:], nrm8[:])
                t1 = wp.tile([128, 8], dt.float32, name="t1", tag="t1", bufs=2)
                nc.vector.tensor_tensor(out=t1[:], in0=y0[:], in1=y0[:], op=ALU.mult)
                nc.vector.tensor_tensor(out=t1[:], in0=t1[:], in1=nsq8[:], op=ALU.mult)
                nc.vector.tensor_scalar(t1[:], t1[:], -0.5, 1.5, op0=ALU.mult, op1=ALU.add)
                nc.vector.tensor_tensor(out=t1[:], in0=t1[:], in1=y0[:], op=ALU.mult)

                sc8 = wp.tile([128, 8], dt.float32, name="sc8", tag="sc8", bufs=2)
                nc.vector.tensor_tensor(out=sc8[:], in0=dot8[:], in1=t1[:], op=ALU.mult)

                mx8 = wp.tile([128, 8], dt.float32, name="mx8", tag="mx8", bufs=2)
                nc.vector.max(out=mx8[:], in_=sc8[:])
                si = wp.tile([128, 8], dt.uint32, name="si", tag="si", bufs=2)
                nc.vector.max_index(out=si[:], in_max=mx8[:], in_values=sc8[:])
                sif = wp.tile([128, 1], dt.float32, name="sif", tag="sif", bufs=2)
                nc.vector.tensor_copy(sif[:], si[:, 0:1])

                for j in range(8):
                    mjf = wp.tile([128, 1], dt.float32, name="mjf", tag="mjf", bufs=2)
                    nc.vector.tensor_scalar(mjf[:], sif[:], float(j), None, op0=ALU.is_equal)
                    nc.vector.scalar_tensor_tensor(out=xbt[:], in0=cands[j][:],
                                                   scalar=mjf[:], in1=xbt[:],
                                                   op0=ALU.mult, op1=ALU.add)
                nc.sync.dma_start(out=out[bt * 128:(bt + 1) * 128, :], in_=xbt[:])

    nc.compile()
    return nc


def make_in_maps(x, memory, B_loc=None, M_=None, D_=None, ncores=NCORES):
    Bfull, D_ = x.shape
    M_, _ = memory.shape
    if B_loc is None:
        B_loc = Bfull // ncores
    BLK = 2048
    NBLK = M_ // BLK
    NCAND = NBLK * 8
    iota = np.broadcast_to(np.arange(NCAND, dtype=np.float32), (128, NCAND)).copy()
    blkoff = np.broadcast_to(
        (np.arange(NCAND, dtype=np.float32) // 8).astype(np.float32) * BLK,
        (128, NCAND)).copy()
    memT = np.ascontiguousarray(memory.T)
    mem = np.ascontiguousarray(memory)
    in_maps = []
    for i in range(ncores):
        xs = np.ascontiguousarray(x[i * B_loc:(i + 1) * B_loc])
        in_maps.append(dict(x=xs, xT=np.ascontiguousarray(xs.T), mem=mem,
                            memT=memT, iota=iota, blkoff=blkoff))
    return in_maps


_cached = {}


def kernel(x, memory):
    x = np.asarray(x, dtype=np.float32)
    memory = np.asarray(memory, dtype=np.float32)
    Bfull, D_ = x.shape
    M_, _ = memory.shape
    B_loc = Bfull // NCORES
    key = (B_loc, M_, D_)
    if key not in _cached:
        nc = build(B_loc=B_loc, M_=M_, D_=D_)
        nc.m = get_hw_module(nc.m)
        _cached[key] = nc
    nc = _cached[key]
    in_maps = make_in_maps(x, memory, B_loc=B_loc, M_=M_, D_=D_)
    res = bass_utils.run_bass_kernel_spmd(nc, in_maps, core_ids=list(range(NCORES)))
    return np.concatenate([res.results[i]["out"] for i in range(NCORES)], axis=0)
